# revision 6
# baseline (speedup 1.0000x reference)
"""KMeans summarize kernel for Trainium2, data-parallel over N on 8 NeuronCores.

v2: bf16 single-pass matmuls, host-packed [XW|W], host XX, paired min-reduce.

Per core (62592 rows = 489 tiles of 128):
  PE:   q = CChat - 2*Xhat@Chat^T via bf16 matmul (stationary [X^T;1;1] tile,
        moving [-2C^T; CC_hi; CC_lo]); CC split keeps the centroid-norm bias
        accurate to ~2^-17 despite bf16.
        scatter: acc[c,k] += sum_i M[i,c]*A[i,k], M=[XW|W] bf16, A in {-1,+1} bf16
  DVE:  m = min_k q per row, two tiles (2 PSUM banks) per reduce
  GPS:  bias b = m + delta
  ACT:  A = Sign(b - q)  -- exactly +1 at the argmin, -1 elsewhere; bf16 out
Host: S = (acc + T)/2 with T = -rowsum(acc)/510 recovered on device;
inertia = sum sqrt(clip(XX+m,0)/64) accumulated on device per partition
(XX = row norms of X, computed host-side).
"""

import sys

sys.path.insert(0, "/opt/trn_rl_repo")

import numpy as np
import ml_dtypes

N, D, K = 500_000, 64, 512
NCORES = 8
P = 128
TILES = 489                      # per-core tiles
ROWS = P * TILES                 # 62592 per core
NPAD = NCORES * ROWS             # 500736
GROUP = 16
DELTA = 2.0 ** -12
_GROUPS = [GROUP] * (TILES // GROUP) + ([TILES % GROUP] if TILES % GROUP else [])

_CACHE = {}


def _build(groups=None):
    import concourse.bass as bass
    import concourse.mybir as mybir
    import concourse.tile as tile

    if groups is None:
        groups = _GROUPS
    tiles = sum(groups)
    assert tiles == TILES

    fp32 = mybir.dt.float32
    f32r = mybir.dt.float32r
    bf16 = mybir.dt.bfloat16
    nc = bass.Bass()
    xt = nc.dram_tensor("xt", (D + 2, ROWS), bf16, kind="ExternalInput")
    mt = nc.dram_tensor("mt", (P, ROWS), f32r, kind="ExternalInput")
    xx = nc.dram_tensor("xx", (P, TILES), fp32, kind="ExternalInput")
    rhs = nc.dram_tensor("rhs", (D + 2, K), bf16, kind="ExternalInput")
    out = nc.dram_tensor("out", (P, K + 4), fp32, kind="ExternalOutput")

    AX = mybir.AxisListType.X
    OP = mybir.AluOpType
    AF = mybir.ActivationFunctionType

    from contextlib import ExitStack

    with tile.TileContext(nc) as tc, ExitStack() as es:
        consts = es.enter_context(tc.tile_pool(name="consts", bufs=1))
        xtp = es.enter_context(tc.tile_pool(name="xtp", bufs=3))
        mgp = es.enter_context(tc.tile_pool(name="mgp", bufs=3))
        ap_ = es.enter_context(tc.tile_pool(name="ap_", bufs=4))
        dpp = es.enter_context(tc.tile_pool(name="dpp", bufs=5, space="PSUM"))
        app = es.enter_context(tc.tile_pool(name="app", bufs=1, space="PSUM"))

        rhs_sb = consts.tile([D + 2, K], bf16)
        nc.sync.dma_start(out=rhs_sb, in_=rhs.ap())
        mbuf = consts.tile([P, tiles], fp32)
        bbuf = consts.tile([P, tiles], fp32)
        xxbuf = consts.tile([P, tiles], fp32)
        nc.sync.dma_start(out=xxbuf, in_=xx.ap())
        scat = app.tile([P, K], fp32)            # scatter accumulator (1 bank)

        t0 = 0
        for g in groups:
            xtg = xtp.tile([D + 2, GROUP * P], bf16, tag="xtg")
            nc.sync.dma_start(
                out=xtg[:, : g * P], in_=xt.ap()[:, t0 * P:(t0 + g) * P]
            )
            mg = mgp.tile([P, GROUP * P], f32r, tag="mg")
            nc.sync.dma_start(
                out=mg[:, : g * P], in_=mt.ap()[:, t0 * P:(t0 + g) * P]
            )
            for j in range(g):
                t = t0 + j
                dp = dpp.tile([P, K], fp32, tag="dp")
                nc.tensor.matmul(
                    dp, xtg[:, j * P:(j + 1) * P],
                    rhs_sb, start=True, stop=True,
                )
                nc.vector.tensor_reduce(
                    out=mbuf[:, t:t + 1], in_=dp, axis=AX, op=OP.min
                )
                nc.gpsimd.tensor_scalar_add(
                    bbuf[:, t:t + 1], mbuf[:, t:t + 1], DELTA
                )
                a_t = ap_.tile([P, K], f32r, tag="a_t")
                nc.scalar.activation(
                    out=a_t, in_=dp, func=AF.Sign,
                    bias=bbuf[:, t:t + 1], scale=-1.0,
                )
                nc.tensor.matmul(
                    scat, mg[:, j * P:(j + 1) * P], a_t,
                    start=(t == 0), stop=(t == tiles - 1),
                )
            t0 += g

        # ---- finalize ----
        out_sb = consts.tile([P, K + 4], fp32)
        t_sb = consts.tile([P, 1], fp32)
        nc.vector.tensor_reduce(out=t_sb, in_=scat, axis=AX, op=OP.add)
        nc.vector.tensor_scalar_mul(t_sb, t_sb, -1.0 / 510.0)
        # S = (scat + T) * 0.5
        nc.vector.tensor_scalar(
            out=out_sb[:, :K], in0=scat, scalar1=t_sb, scalar2=0.5,
            op0=OP.add, op1=OP.mult,
        )
        # inertia partials: sum sqrt(clip(m+xx,0)/64)
        tt_b = consts.tile([P, tiles], fp32)
        nc.vector.tensor_tensor(out=tt_b, in0=mbuf, in1=xxbuf, op=OP.add)
        nc.vector.tensor_scalar_max(tt_b, tt_b, 0.0)
        sq_b = consts.tile([P, tiles], fp32)
        inert = consts.tile([P, 1], fp32)
        nc.scalar.activation(
            out=sq_b, in_=tt_b, func=AF.Sqrt, scale=1.0 / D, accum_out=inert
        )
        nc.vector.tensor_copy(out_sb[:, K:K + 1], inert)
        nc.vector.tensor_copy(out_sb[:, K + 1:K + 2], t_sb)
        nc.sync.dma_start(out=out.ap(), in_=out_sb)

    _split_multi_waits(nc, mybir)
    return nc


def _split_multi_waits(nc, mybir):
    """This walrus build allows max 1 sem-wait per instruction: hoist extras
    onto inserted NoOps on the same engine queue."""
    import copy

    module = nc.m
    new_module = copy.replace(module, functions=[])
    for function in module.functions:
        new_function = copy.replace(function, blocks=[])
        new_function.set_allocations_from_list(function.allocations)
        for block in function.blocks:
            new_insts = []
            for ins in block.instructions:
                si = ins.sync_info
                if si is not None and si.on_wait and len(si.on_wait) > 1:
                    waits = list(si.on_wait)
                    for k, w in enumerate(waits[:-1]):
                        new_insts.append(mybir.InstNoOp(
                            name=f"{ins.name}-wsplit{k}", engine=ins.engine,
                            ins=[], outs=[],
                            sync_info=mybir.SyncInfo(on_wait=[w], on_update=[]),
                        ))
                    ins.sync_info = mybir.SyncInfo(
                        on_wait=[waits[-1]], on_update=list(si.on_update or [])
                    )
                new_insts.append(ins)
            new_function.blocks.append(copy.replace(block, instructions=new_insts))
        new_module.functions.append(new_function)
    nc.m = new_module


def _prep_inputs(X, centroids, sample_weight):
    bf = ml_dtypes.bfloat16
    C = np.asarray(centroids, dtype=np.float32)
    Xp = np.empty((NPAD, D), dtype=np.float32)
    Xp[:N] = X
    Xp[N:] = C[0]
    Wp = np.zeros((NPAD, D), dtype=np.float32)
    Wp[:N] = sample_weight

    CC = (C.astype(np.float64) ** 2).sum(axis=1).astype(np.float32)
    CCh = CC.astype(bf).astype(np.float32)
    rhs = np.empty((D + 2, K), dtype=np.float32)
    rhs[:D] = -2.0 * C.T
    rhs[D] = CCh
    rhs[D + 1] = CC - CCh
    rhs_b = np.ascontiguousarray(rhs.astype(bf))

    XX = np.einsum("ij,ij->i", Xp.astype(np.float64), Xp.astype(np.float64))
    XX = XX.astype(np.float32)
    Xb = Xp.astype(bf)
    XWb = Xb.astype(np.float32) * Wp
    Wb = Wp

    in_maps = []
    for c in range(NCORES):
        sl = slice(c * ROWS, (c + 1) * ROWS)
        xt_a = np.empty((D + 2, ROWS), dtype=bf)
        xt_a[:D] = Xb[sl].T
        xt_a[D:] = np.float32(1.0)
        m = np.concatenate([XWb[sl], Wb[sl]], axis=1)       # [ROWS, 128] fp32
        mt_a = np.ascontiguousarray(
            m.reshape(TILES, P, 2 * D).transpose(1, 0, 2).reshape(P, ROWS)
        )
        xx_a = np.ascontiguousarray(XX[sl].reshape(TILES, P).T)
        in_maps.append({
            "xt": np.ascontiguousarray(xt_a), "mt": mt_a,
            "xx": xx_a, "rhs": rhs_b,
        })
    return in_maps


def run(X, centroids, sample_weight, trace=False):
    from concourse.bass_utils import run_bass_kernel_spmd

    if "nc" not in _CACHE:
        _CACHE["nc"] = _build()
    in_maps = _prep_inputs(X, centroids, sample_weight)
    res = run_bass_kernel_spmd(
        _CACHE["nc"], in_maps, core_ids=list(range(NCORES)), trace=trace
    )
    xw = np.zeros((K, D), dtype=np.float64)
    ws = np.zeros((K, D), dtype=np.float64)
    inertia = 0.0
    for c in range(NCORES):
        o = res.results[c]["out"]
        xw += o[:D, :K].T.astype(np.float64)
        ws += o[D:2 * D, :K].T.astype(np.float64)
        inertia += float(o[:, K].sum(dtype=np.float64))
    packed = np.concatenate(
        [xw, ws, np.full((1, D), inertia)], axis=0
    ).astype(np.float32)
    return packed, res


def kernel(X, centroids, sample_weight):
    packed, _ = run(X, centroids, sample_weight)
    return packed
